# revision 1
# baseline (speedup 1.0000x reference)
"""Cosine multi-head attention (h=1) Trainium2 kernel.

Math (reference):
    context = query @ Wq.T + bq                  [B, S, HD]
    ctx     = context * weight_tensor[0]         (elementwise over HD)
    ctx_n   = ctx / max(||ctx||_2, eps)          (normalize over HD)
    scores  = ctx_n @ ctx_n.T                    [B, S, S]
    out     = softmax(scores, axis=-1)

Device strategy (8 cores, SPMD):
    core c handles batch b = c//2, row-half h = c%2.  The host rotates the
    batch's rows so each core's own 2048 rows come first, transposes to
    qT [D, S] (so the device never needs an on-chip transpose of q), splits
    it into bf16 hi/lo halves (q = hi + lo to ~2^-17 relative), and folds
    weight_tensor into Wq:  M = diag(w) @ Wq (also hi/lo),  c0 = w * bq.

    All matmuls run in bf16 with the 3-term compensated product
    A·B = Ahi·Bhi + Ahi·Blo + Alo·Bhi  (the lo·lo term is ~2^-34 and
    dropped) — native fp32 matmul on TRN2 is a 2-pass LOW_HIGH mode that
    is ~5.5x slower per element than bf16.

    On device:
      CT[hd, s] = sum_d M[hd, d] qT[d, s]          (PSUM-resident [120, 4096])
      ct_sb = CT + c0  (bias folded into the ACT PSUM->SBUF copy, c0 is
                        per-partition in this layout)
      norm2 broadcast over partitions via ones-matmul;
      inv_norm = exp(-0.5 * ln(max(norm2, eps^2)))  (ACT ln+exp; DVE
                reciprocal at 8 cyc/elem would cost ~25 us)
      Cn = ct_sb * inv_norm; split Cn into bf16 hi/lo
      per 128-row chunk i of the first 2048 rows:
         R = Cn[:, i-chunk].T @ Cn                  (PE, 3-term bf16, PSUM)
         E = exp(R) with fused row-sum (ACT accum_out)
         out_rows = E * (1/rowsum)                  (GPSIMD; DVE only does
                the tiny [128,1] reciprocal) -> DMA out
    Softmax needs no max-subtraction: scores are cosines in [-1, 1].

    Output columns of h=1 cores are rotated by 2048; the host gather undoes it.
"""

import numpy as np
from contextlib import ExitStack

B, S, D, HD = 4, 4096, 1024, 120
ROWS = S // 2  # rows of the score matrix each core produces
EPS = 1e-12
N_CORES = 8

_NC_CACHE = {}


def _build_nc():
    import concourse.bacc as bacc
    import concourse.tile as tile
    from concourse import mybir

    f32 = mybir.dt.float32
    bf16 = mybir.dt.bfloat16
    AF = mybir.ActivationFunctionType
    nc = bacc.Bacc("TRN2", target_bir_lowering=False, debug=False,
                   num_devices=N_CORES)

    q2 = nc.declare_dram_parameter("q2", [2 * D, S], bf16, isOutput=False)
    mt_hi = nc.declare_dram_parameter("mt_hi", [D, HD], bf16, isOutput=False)
    mt_lo = nc.declare_dram_parameter("mt_lo", [D, HD], bf16, isOutput=False)
    c0 = nc.declare_dram_parameter("c0", [HD, 1], f32, isOutput=False)
    out = nc.declare_dram_parameter("out", [ROWS, S], f32, isOutput=True)

    DC = D // 128   # 8 contraction chunks
    KC = S // 512   # 8 column groups of 512

    with ExitStack() as ctx:
        tc = ctx.enter_context(tile.TileContext(nc))
        singles = ctx.enter_context(tc.tile_pool(name="singles", bufs=1))
        qpool = ctx.enter_context(tc.tile_pool(name="qpool", bufs=3))
        work = ctx.enter_context(tc.tile_pool(name="work", bufs=1))
        epool = ctx.enter_context(tc.tile_pool(name="epool", bufs=3))
        spool = ctx.enter_context(tc.tile_pool(name="spool", bufs=4))
        ps = ctx.enter_context(tc.tile_pool(name="ps", bufs=2, space="PSUM"))

        # --- phases 1+2, streamed by 2048-column half so half A's norm/
        #     normalize chain overlaps half B's input DMA.  q2 stacks the
        #     bf16 hi and lo halves of qT; DMAs fetch 2 d-chunks x {hi,lo}
        #     x 2048 cols = 2 MB at a time.
        #     view: row = t*1024 + c*128 + p, col = h*2048 + j
        q2_r = q2.rearrange("(t cp c2 p) (h j) -> cp h p c2 t j",
                            t=2, cp=DC // 2, c2=2, p=128, h=2)
        mth_sb = mtl_sb = c0_sb = ones_sq = None
        # constants first in the DMA queue (tiny; the c==0 matmuls need mth)
        mth_sb = singles.tile([128, DC, HD], bf16, tag="mth")
        nc.sync.dma_start(out=mth_sb[:],
                          in_=mt_hi.rearrange("(c p) h -> p c h", p=128))
        mtl_sb = singles.tile([128, DC, HD], bf16, tag="mtl")
        nc.sync.dma_start(out=mtl_sb[:],
                          in_=mt_lo.rearrange("(c p) h -> p c h", p=128))
        c0_sb = singles.tile([HD, 1], f32, tag="c0")
        nc.sync.dma_start(out=c0_sb[:], in_=c0[:])
        ones_sq = singles.tile([HD, HD], f32, tag="ones_sq")
        nc.vector.memset(ones_sq[:], 1.0)

        cn_hi = []
        cn_lo = []
        half_state = []
        for half in range(2):
            ct_h_ps = ps.tile([HD, 2048], f32, tag="ps4", name=f"ct_ps{half}")
            for cp in range(DC // 2):
                q2c = qpool.tile([128, 2, 2, 2048], bf16, tag="q2",
                                 name=f"q2_{half}_{cp}")
                for t in range(2):
                    nc.sync.dma_start(out=q2c[:, :, t, :],
                                      in_=q2_r[cp, half, :, :, t, :])
                for c2 in range(2):
                    c = cp * 2 + c2
                    for k in range(4):
                        sl = ct_h_ps[:, k * 512:(k + 1) * 512]
                        jsl = slice(k * 512, (k + 1) * 512)
                        nc.tensor.matmul(sl, lhsT=mth_sb[:, c, :],
                                         rhs=q2c[:, c2, 0, jsl],
                                         start=(c == 0), stop=False)
                        nc.tensor.matmul(sl, lhsT=mth_sb[:, c, :],
                                         rhs=q2c[:, c2, 1, jsl],
                                         start=False, stop=False)
                    for k in range(4):
                        sl = ct_h_ps[:, k * 512:(k + 1) * 512]
                        jsl = slice(k * 512, (k + 1) * 512)
                        nc.tensor.matmul(sl, lhsT=mtl_sb[:, c, :],
                                         rhs=q2c[:, c2, 0, jsl],
                                         start=False, stop=(c == DC - 1))

            # ACT copy + DVE square run as soon as this half's psum is done;
            # the norm matmuls are EMITTED after both halves' ct matmuls so
            # the in-order PE queue never stalls at a half boundary.
            ct_h = work.tile([HD, 2048], f32, tag=f"ct{half}", name=f"ct{half}")
            nc.scalar.activation(out=ct_h[:], in_=ct_h_ps[:],
                                 func=AF.Identity, bias=c0_sb[:], scale=1.0)
            ctsq = work.tile([HD, 2048], f32, tag=f"ctsq{half}",
                             name=f"ctsq{half}")
            nc.vector.tensor_mul(ctsq[:], ct_h[:], ct_h[:])
            half_state.append((ct_h, ctsq))

        n_pss = []
        for half in range(2):
            ct_h, ctsq = half_state[half]
            n_ps = ps.tile([HD, 2048], f32, tag="ps4", name=f"n_ps{half}")
            for k in range(4):
                nc.tensor.matmul(n_ps[:, k * 512:(k + 1) * 512],
                                 lhsT=ones_sq[:],
                                 rhs=ctsq[:, k * 512:(k + 1) * 512],
                                 start=True, stop=True)
            n_pss.append(n_ps)

        for half in range(2):
            ct_h, ctsq = half_state[half]
            n_ps = n_pss[half]
            # clamp + rsqrt in place in PSUM (saves two SBUF tiles)
            nc.vector.tensor_scalar_max(n_ps[:], n_ps[:], EPS * EPS)
            # single-op rsqrt: 1/sqrt(|x|); input already clamped positive
            nc.scalar.activation(out=n_ps[:], in_=n_ps[:],
                                 func=AF.Abs_reciprocal_sqrt)
            # cn reuses ctsq's slot (ctsq is dead after the norm matmuls)
            cn_h = work.tile([HD, 2048], f32, tag=f"ctsq{half}",
                             name=f"cn{half}")
            nc.vector.tensor_mul(cn_h[:], ct_h[:], n_ps[:])
            hi_h = work.tile([HD, 2048], bf16, tag=f"cnh{half}",
                             name=f"cnh{half}")
            nc.vector.tensor_copy(hi_h[:], cn_h[:])
            lo_h = work.tile([HD, 2048], bf16, tag=f"cnl{half}",
                             name=f"cnl{half}")
            nc.vector.tensor_sub(lo_h[:], cn_h[:], hi_h[:])
            cn_hi.append(hi_h)
            cn_lo.append(lo_h)

        def rhs_hi(k):  # [120, 512] bf16 slice of Cn_hi, k in 0..7
            return cn_hi[k // 4][:, (k % 4) * 512:(k % 4 + 1) * 512]

        def rhs_lo(k):
            return cn_lo[k // 4][:, (k % 4) * 512:(k % 4 + 1) * 512]

        # --- phase 3: gram + softmax; pairs of 128-row chunks share an
        #     output tile so DMA-out goes in 4 MB transfers ---
        NCHUNK = ROWS // 128
        for i in range(NCHUNK):
            ic = i % 2
            if ic == 0:
                e2 = epool.tile([128, 2, S], f32, tag="e", name=f"e{i}")
                sums = spool.tile([128, 4], f32, tag="sums", name=f"sums{i}")
            hcol = (i * 128) // 2048
            off = (i * 128) % 2048
            hi_i = cn_hi[hcol][:, off:off + 128]
            lo_i = cn_lo[hcol][:, off:off + 128]
            for jg in range(2):
                r_ps = ps.tile([128, 2048], f32, tag="ps4",
                               name=f"r_ps{i}_{jg}")
                for k in range(4):
                    kk = jg * 4 + k
                    nc.tensor.matmul(r_ps[:, k * 512:(k + 1) * 512],
                                     lhsT=hi_i, rhs=rhs_hi(kk),
                                     start=True, stop=False)
                    nc.tensor.matmul(r_ps[:, k * 512:(k + 1) * 512],
                                     lhsT=hi_i, rhs=rhs_lo(kk),
                                     start=False, stop=False)
                for k in range(4):
                    kk = jg * 4 + k
                    nc.tensor.matmul(r_ps[:, k * 512:(k + 1) * 512],
                                     lhsT=lo_i, rhs=rhs_hi(kk),
                                     start=False, stop=True)
                nc.scalar.activation(
                    out=e2[:, ic, jg * 2048:(jg + 1) * 2048],
                    in_=r_ps[:],
                    func=AF.Exp,
                    accum_out=sums[:, 2 * ic + jg:2 * ic + jg + 1],
                )
            tot = spool.tile([128, 1], f32, tag="tot", name=f"tot{i}")
            nc.vector.tensor_add(tot[:], sums[:, 2 * ic:2 * ic + 1],
                                 sums[:, 2 * ic + 1:2 * ic + 2])
            rec = spool.tile([128, 1], f32, tag="rec", name=f"rec{i}")
            nc.vector.reciprocal(rec[:], tot[:])
            nc.vector.tensor_scalar_mul(e2[:, ic, :], e2[:, ic, :], rec[:])
            if i >= NCHUNK - 2:
                # drain the tail in single-chunk DMAs (shorter critical path)
                nc.sync.dma_start(out=out[i * 128:(i + 1) * 128, :],
                                  in_=e2[:, ic, :])
            elif ic == 1:
                nc.sync.dma_start(
                    out=out[(i - 1) * 128:(i + 1) * 128, :].rearrange(
                        "(c p) s -> p c s", p=128),
                    in_=e2[:],
                )

    nc.compile()
    return nc


def _get_nc():
    if "nc" not in _NC_CACHE:
        _NC_CACHE["nc"] = _build_nc()
    return _NC_CACHE["nc"]


def _split_hi_lo(a32):
    import ml_dtypes
    hi = a32.astype(ml_dtypes.bfloat16)
    lo = (a32 - hi.astype(np.float32)).astype(ml_dtypes.bfloat16)
    return np.ascontiguousarray(hi), np.ascontiguousarray(lo)


def _make_in_maps(inputs):
    query = np.asarray(inputs["query"], dtype=np.float32)
    Wq = np.asarray(inputs["Wq"], dtype=np.float32)
    bq = np.asarray(inputs["bq"], dtype=np.float32)
    w = np.asarray(inputs["weight_tensor"], dtype=np.float32)

    w0 = w.reshape(-1)[:HD]
    mt_hi, mt_lo = _split_hi_lo((w0[:, None] * Wq).T)           # [D, HD]
    c0_np = np.ascontiguousarray((w0 * bq)[:, None])            # [HD, 1]

    in_maps = []
    for c in range(N_CORES):
        b, h = c // 2, c % 2
        qb = query[b]
        if h:
            qb = np.concatenate([qb[ROWS:], qb[:ROWS]], axis=0)
        q_hi, q_lo = _split_hi_lo(qb.T)
        q2_np = np.ascontiguousarray(np.concatenate([q_hi, q_lo], axis=0))
        in_maps.append({"q2": q2_np, "mt_hi": mt_hi,
                        "mt_lo": mt_lo, "c0": c0_np})
    return in_maps


def _gather(results):
    full = np.empty((B, S, S), dtype=np.float32)
    for c in range(N_CORES):
        b, h = c // 2, c % 2
        r = results[c]["out"]
        if h == 0:
            full[b, :ROWS] = r
        else:
            full[b, ROWS:, ROWS:] = r[:, :ROWS]
            full[b, ROWS:, :ROWS] = r[:, ROWS:]
    return full


def kernel(**inputs):
    from concourse.bass_utils import run_bass_kernel_spmd

    in_maps = _make_in_maps(inputs)
    nc = _get_nc()
    res = run_bass_kernel_spmd(nc, in_maps, list(range(N_CORES))).results
    return _gather(res)


def _register_ntff_hook():
    """Register the axon NTFF profile hook that the agent image's antenv
    package lacks (see trn_boot.py) so trace=True yields exec_time_ns."""
    import sys
    import types
    try:
        import antenv.axon_hooks  # noqa: F401
        return True
    except ImportError:
        pass
    try:
        from trn_agent_boot.trn_boot import _ntff_profile_via_ctypes
        hook = _ntff_profile_via_ctypes("/opt/axon/libaxon_pjrt.so")
    except Exception:
        return False
    if hook is None:
        return False
    mod = types.ModuleType("antenv.axon_hooks")
    mod._hook = hook
    mod.get_axon_ntff_profile_hook = lambda: mod._hook
    mod.set_axon_ntff_profile_hook = lambda h: setattr(mod, "_hook", h)
    sys.modules["antenv.axon_hooks"] = mod
    import antenv
    antenv.axon_hooks = mod
    return True


def profile_once(inputs, trace_cores=None):
    """Re-run the kernel with NTFF profiling; returns max exec_time_ns."""
    import tempfile
    import concourse.bass_utils as bu

    _register_ntff_hook()
    # avoid the cloud artifact upload inside the trace path
    bu.upload_artifacts = lambda tmpdir: tmpdir

    in_maps = _make_in_maps(inputs)
    nc = _get_nc()
    tmpdir = tempfile.mkdtemp(prefix="ntff_")
    r = bu.run_bass_kernel_spmd(nc, in_maps, list(range(N_CORES)),
                                trace=True, trace_cores=trace_cores,
                                tmpdir=tmpdir)
    print(f"trace dir: {tmpdir}")
    if r.exec_time_ns is not None:
        print(f"mean exec: {r.mean_exec_time_ns} ns, "
              f"max core: {r.max_exec_time_core_id}")
    return r.exec_time_ns



# revision 5
# speedup vs baseline: 1.4276x; 1.4276x over previous
"""Cosine multi-head attention (h=1) Trainium2 kernel, v2.

Math (reference):
    context = query @ Wq.T + bq                  [B, S, HD]
    ctx     = context * weight_tensor[0]         (elementwise over HD)
    ctx_n   = ctx / max(||ctx||_2, eps)          (normalize over HD)
    scores  = ctx_n @ ctx_n.T                    [B, S, S]
    out     = softmax(scores, axis=-1)

Device strategy (8 cores, SPMD): core c handles batch b = c//2, row-half
h = c%2.  The host rotates the batch's rows so each core's own 2048 rows
come first and ships qT [D, S] in fp16; weight_tensor is folded into Wq
(M = diag(w) @ Wq, fp16) and c0 = w * bq.

Precision budget: the harness gate is rel_err < 2e-2 (abs-max / max|ref|).
A pure-fp16 pipeline (fp16 matmul inputs, fp32 PSUM accum) measures
~6e-4; quantizing the softmax numerator to uint8 adds ~2e-3.  Total
~3e-3, 7x inside the gate, while tripling matmul speed vs the fp32r /
compensated-bf16 alternatives and halving output DMA bytes.

On device:
    CT[hd, s] = sum_d M[hd, d] qT[d, s]       (PSUM [120, 2048] per half)
    ct = CT + c0 (DVE), sq = ct^2 (DVE, fp16)
    n  = ones[120,120] @ sq                   (PE broadcast-sum, PSUM)
    inv = exp(-0.5 * ln n)  (ACT; ln and exp share one ACT table set, so
          the whole kernel runs off a single table load — Abs_reciprocal
          _sqrt would force two table swaps per half mid-exp-stream)
    cn = ct * inv -> fp16                     (DVE)
    per 128-row chunk i, col-half jg:
        R = cn[:, i-chunk].T @ cn[:, jg]      (PE, fp16, PSUM [128, 2048])
        u8 = exp(R + (ln(250) - 1)), row-sums via accum_out   (ACT)
    The exp bias maps scores [-1, 1] -> (33.6, 250.5); the uint8 cast IS
    the output write.  Softmax's division by the row-sum commutes with
    the dequant cast, so the host applies it while widening u8 -> f32:
    out = u8 * (1/rowsum); the 250/e scale cancels exactly since the
    shipped row-sums carry the same factor.

PSUM (8 banks) is time-shared through one rotating 2-slot pool tag
(4 banks per [128, 2048] f32 slot).  Slot order interleaves half-B's
projection/norm tiles between early gram tiles so half-B's compute can
overlap the jg0 exp stream without stalling it.

Output columns of h=1 cores are rotated by 2048; the host gather undoes
it during dequantization.
"""

import numpy as np
from contextlib import ExitStack

B, S, D, HD = 4, 4096, 1024, 120
ROWS = S // 2  # rows of the score matrix each core produces
N_CORES = 8
EXP_BIAS = float(np.log(250.0) - 1.0)

_NC_CACHE = {}


def _build_nc():
    import concourse.bacc as bacc
    import concourse.tile as tile
    from concourse import mybir

    f32 = mybir.dt.float32
    fp16 = mybir.dt.float16
    u8 = mybir.dt.uint8
    AF = mybir.ActivationFunctionType
    nc = bacc.Bacc("TRN2", target_bir_lowering=False, debug=False,
                   num_devices=N_CORES)

    qt = nc.declare_dram_parameter("qt", [D, S], fp16, isOutput=False)
    mt = nc.declare_dram_parameter("mt", [D, HD], fp16, isOutput=False)
    c0 = nc.declare_dram_parameter("c0", [HD, 1], f32, isOutput=False)
    ones = nc.declare_dram_parameter("ones", [HD, HD], fp16, isOutput=False)
    eo = nc.declare_dram_parameter("eo", [ROWS, S], u8, isOutput=True)
    sums = nc.declare_dram_parameter("sums", [128, 2 * (ROWS // 128)], f32,
                                     isOutput=True)

    DC = D // 128   # 8 contraction chunks
    NCHUNK = ROWS // 128  # 16 gram row-chunks

    with ExitStack() as ctx:
        tc = ctx.enter_context(tile.TileContext(nc))
        singles = ctx.enter_context(tc.tile_pool(name="singles", bufs=1))
        qpool = ctx.enter_context(tc.tile_pool(name="qpool", bufs=4))
        work = ctx.enter_context(tc.tile_pool(name="work", bufs=1))
        epool = ctx.enter_context(tc.tile_pool(name="epool", bufs=4))
        ps = ctx.enter_context(tc.tile_pool(name="ps", bufs=2, space="PSUM"))

        # constants first in the DMA queue
        mt_sb = singles.tile([128, DC, HD], fp16, tag="mt")
        nc.sync.dma_start(out=mt_sb[:],
                          in_=mt.rearrange("(c p) h -> p c h", p=128))
        c0_sb = singles.tile([HD, 1], f32, tag="c0")
        nc.sync.dma_start(out=c0_sb[:], in_=c0[:])
        ones_sb = singles.tile([HD, HD], fp16, tag="ones")
        nc.sync.dma_start(out=ones_sb[:], in_=ones[:])

        cn = singles.tile([HD, S], fp16, tag="cn")      # both halves
        sums_sb = singles.tile([128, 2 * NCHUNK], f32, tag="sums")
        ebias = singles.tile([128, 1], f32, tag="ebias")
        nc.vector.memset(ebias[:], EXP_BIAS)

        def load_half(half):
            tiles = []
            for c in range(DC):
                q_sb = qpool.tile([128, ROWS], fp16, tag="q",
                                  name=f"q_{half}_{c}")
                nc.sync.dma_start(
                    out=q_sb[:],
                    in_=qt[c * 128:(c + 1) * 128,
                           half * ROWS:(half + 1) * ROWS])
                tiles.append(q_sb)
            return tiles

        def proj_half(half, q_tiles):
            """PE: CT psum tile for one 2048-col half."""
            ct_ps = ps.tile([HD, ROWS], f32, tag="ps4", name=f"ct_ps{half}")
            for c in range(DC):
                for k in range(4):
                    sl = ct_ps[:, k * 512:(k + 1) * 512]
                    nc.tensor.matmul(sl, lhsT=mt_sb[:, c, :],
                                     rhs=q_tiles[c][:, k * 512:(k + 1) * 512],
                                     start=(c == 0), stop=(c == DC - 1))
            return ct_ps

        def norm_half(half, ct_ps):
            """DVE bias+square, PE ones-matmul, ACT ln+exp, DVE cn mul.
            Emits in 1024-col sub-blocks to shorten the serial tail."""
            ct_sb = work.tile([HD, ROWS], f32, tag=f"ct{half}",
                              name=f"ct{half}")
            ctsq = work.tile([HD, ROWS], fp16, tag=f"ctsq{half}",
                             name=f"ctsq{half}")
            for s2 in range(2):
                sl = slice(s2 * 1024, (s2 + 1) * 1024)
                nc.vector.tensor_scalar_add(ct_sb[:, sl], ct_ps[:, sl],
                                            c0_sb[:])
                nc.vector.tensor_mul(ctsq[:, sl], ct_sb[:, sl], ct_sb[:, sl])
            n_ps = ps.tile([HD, ROWS], f32, tag="ps4", name=f"n_ps{half}")
            for k in range(4):
                nc.tensor.matmul(n_ps[:, k * 512:(k + 1) * 512],
                                 lhsT=ones_sb[:],
                                 rhs=ctsq[:, k * 512:(k + 1) * 512],
                                 start=True, stop=True)
            inv = work.tile([HD, ROWS], f32, tag=f"inv{half}",
                            name=f"inv{half}")
            for s2 in range(2):
                sl = slice(s2 * 1024, (s2 + 1) * 1024)
                nc.scalar.activation(out=n_ps[:, sl], in_=n_ps[:, sl],
                                     func=AF.Ln)
                nc.scalar.activation(out=inv[:, sl], in_=n_ps[:, sl],
                                     func=AF.Exp, scale=-0.5)
                nc.vector.tensor_mul(
                    cn[:, half * ROWS + s2 * 1024:half * ROWS + (s2 + 1) * 1024],
                    ct_sb[:, sl], inv[:, sl])

        def gram_chunk(i, jg):
            """PE gram psum + ACT exp->u8 with row-sum accum + DMA out."""
            r_ps = ps.tile([128, 2048], f32, tag="ps4", name=f"r_ps{i}_{jg}")
            hi = cn[:, i * 128:(i + 1) * 128]
            for k in range(4):
                kk = jg * 4 + k
                nc.tensor.matmul(r_ps[:, k * 512:(k + 1) * 512],
                                 lhsT=hi, rhs=cn[:, kk * 512:(kk + 1) * 512],
                                 start=True, stop=True)
            e_sb = epool.tile([128, 2048], u8, tag="e", name=f"e{i}_{jg}")
            nc.scalar.activation(
                out=e_sb[:], in_=r_ps[:], func=AF.Exp, bias=ebias[:],
                accum_out=sums_sb[:, 2 * i + jg:2 * i + jg + 1])
            nc.sync.dma_start(
                out=eo[i * 128:(i + 1) * 128, jg * 2048:(jg + 1) * 2048],
                in_=e_sb[:])

        # --- half A: load, project, normalize ---
        qa = load_half(0)
        qb = load_half(1)       # queued behind half A's DMAs
        ct_a = proj_half(0, qa)
        norm_half(0, ct_a)      # psum slot T2 (n_psA)

        # --- gram/exp stream with half-B phase-1/2 interleaved so its
        #     psum slots come up for rotation at the right time ---
        gram_chunk(0, 0)        # T1 (after ct_A freed by bias-add)
        gram_chunk(1, 0)        # T2 (after n_A freed by the inv exp)
        gram_chunk(2, 0)        # T1
        ct_b = proj_half(1, qb)  # T2; projection is DMA-paced anyway
        gram_chunk(3, 0)        # T1
        norm_half(1, ct_b)      # T2 (n_psB); ACT ln/exp slot in between exps
        for i in range(4, NCHUNK):
            gram_chunk(i, 0)
        for i in range(NCHUNK):
            gram_chunk(i, 1)

        nc.sync.dma_start(out=sums[:], in_=sums_sb[:])

    nc.compile()
    return nc


def _get_nc():
    if "nc" not in _NC_CACHE:
        _NC_CACHE["nc"] = _build_nc()
    return _NC_CACHE["nc"]


def _make_in_maps(inputs):
    query = np.asarray(inputs["query"], dtype=np.float32)
    Wq = np.asarray(inputs["Wq"], dtype=np.float32)
    bq = np.asarray(inputs["bq"], dtype=np.float32)
    w = np.asarray(inputs["weight_tensor"], dtype=np.float32)

    w0 = w.reshape(-1)[:HD]
    mt_np = np.ascontiguousarray((w0[:, None] * Wq).T.astype(np.float16))
    c0_np = np.ascontiguousarray((w0 * bq)[:, None].astype(np.float32))
    ones_np = np.ones((HD, HD), dtype=np.float16)

    in_maps = []
    for c in range(N_CORES):
        b, h = c // 2, c % 2
        qb = query[b]
        if h:
            qb = np.concatenate([qb[ROWS:], qb[:ROWS]], axis=0)
        qt_np = np.ascontiguousarray(qb.T.astype(np.float16))
        in_maps.append({"qt": qt_np, "mt": mt_np, "c0": c0_np,
                        "ones": ones_np})
    return in_maps


def _gather(results):
    full = np.empty((B, S, S), dtype=np.float32)
    for c in range(N_CORES):
        b, h = c // 2, c % 2
        r = results[c]["eo"]
        sm = results[c]["sums"]
        # sums[p, 2i+jg] holds row i*128+p's partial sum for col-half jg
        s = sm.reshape(128, ROWS // 128, 2).sum(axis=2).T.reshape(ROWS)
        rec = (1.0 / s).astype(np.float32)
        deq = r.astype(np.float32) * rec[:, None]
        if h == 0:
            full[b, :ROWS] = deq
        else:
            full[b, ROWS:, ROWS:] = deq[:, :ROWS]
            full[b, ROWS:, :ROWS] = deq[:, ROWS:]
    return full


def kernel(**inputs):
    from concourse.bass_utils import run_bass_kernel_spmd

    in_maps = _make_in_maps(inputs)
    nc = _get_nc()
    res = run_bass_kernel_spmd(nc, in_maps, list(range(N_CORES))).results
    return _gather(res)


def _register_ntff_hook():
    """Register the axon NTFF profile hook that the agent image's antenv
    package lacks (see trn_boot.py) so trace=True yields exec_time_ns."""
    import sys
    import types
    try:
        import antenv.axon_hooks  # noqa: F401
        return True
    except ImportError:
        pass
    try:
        from trn_agent_boot.trn_boot import _ntff_profile_via_ctypes
        hook = _ntff_profile_via_ctypes("/opt/axon/libaxon_pjrt.so")
    except Exception:
        return False
    if hook is None:
        return False
    mod = types.ModuleType("antenv.axon_hooks")
    mod._hook = hook
    mod.get_axon_ntff_profile_hook = lambda: mod._hook
    mod.set_axon_ntff_profile_hook = lambda h: setattr(mod, "_hook", h)
    sys.modules["antenv.axon_hooks"] = mod
    import antenv
    antenv.axon_hooks = mod
    return True


def profile_once(inputs, trace_cores=None):
    """Re-run the kernel with NTFF profiling; returns max exec_time_ns."""
    import tempfile
    import concourse.bass_utils as bu

    _register_ntff_hook()
    # avoid the cloud artifact upload inside the trace path
    bu.upload_artifacts = lambda tmpdir: tmpdir

    in_maps = _make_in_maps(inputs)
    nc = _get_nc()
    tmpdir = tempfile.mkdtemp(prefix="ntff_")
    r = bu.run_bass_kernel_spmd(nc, in_maps, list(range(N_CORES)),
                                trace=True, trace_cores=trace_cores,
                                tmpdir=tmpdir)
    print(f"trace dir: {tmpdir}")
    if r.exec_time_ns is not None:
        print(f"mean exec: {r.mean_exec_time_ns} ns, "
              f"max core: {r.max_exec_time_core_id}")
    return r.exec_time_ns


# revision 6
# speedup vs baseline: 1.6802x; 1.1769x over previous
"""Cosine multi-head attention (h=1) Trainium2 kernel, v3.

Math (reference):
    context = query @ Wq.T + bq                  [B, S, HD]
    ctx     = context * weight_tensor[0]         (elementwise over HD)
    ctx_n   = ctx / max(||ctx||_2, eps)          (normalize over HD)
    scores  = ctx_n @ ctx_n.T                    [B, S, S]
    out     = softmax(scores, axis=-1)

Device strategy (8 cores, SPMD): core c handles batch b = c//2, row-half
h = c%2.  The host rotates the batch's rows so each core's own 2048 rows
come first and ships qT [D, S] in fp16; weight_tensor is folded into Wq
(M = diag(w) @ Wq, fp16) and c0 = w * bq.

Precision budget: the harness gate is rel_err < 2e-2 (abs-max / max|ref|).
A pure-fp16 pipeline (fp16 matmul inputs, fp32 PSUM accum) measures
~6e-4; quantizing the softmax numerator to uint8 adds ~2e-3.  Total
~3e-3, 7x inside the gate, while tripling matmul speed vs the fp32r /
compensated-bf16 alternatives and halving output DMA bytes.

On device:
    CT[hd, s] = sum_d M[hd, d] qT[d, s]       (PSUM [120, 2048] per half)
    ct = (CT + c0) -> fp16 (DVE), sq = ct^2 (DVE 2x)
    n  = ones[120,120] @ sq                   (PE broadcast-sum, PSUM)
    inv = n^-1/2 -> fp16   (one ACT Abs_reciprocal_sqrt per half)
    cn = ct * inv -> fp16                     (DVE 2x)
    per 128-row chunk i, col-half jg:
        R = cn[:, i-chunk].T @ cn[:, jg]      (PE, fp16, PSUM [128, 2048])
        u8 = exp(R + (ln(250) - 1))           (ACT, no accumulator)
    The exp bias maps scores [-1, 1] -> (33.6, 250.5); the uint8 cast IS
    the output write.  No row-sum is computed on device: softmax
    self-normalizes from the quantized numerators on the host,
    out = u8 / sum_j(u8) (the 250/e scale cancels; the quantization
    error this adds to the denominator is ~1e-4 relative).

Hardware lessons this version encodes (from NTFF traces):
  - one dma_start lands on ONE of the 16 DMA queues (~22.5 GB/s each),
    so every bulk transfer is chopped into [128, 512] pieces that
    round-robin across queues (4.2 MB half-q arrives in ~12 us, not 23).
  - ACT accum_out costs ~0.95 us per instruction on HW (vs 187 ns in
    the cost model) -- dropped entirely (see above).
  - Ln and Exp do NOT share an ACT table set on this toolchain: each
    switch is a 1.28 us table load.  One rsqrt per half keeps it to 4
    loads total (Sqrt/Exp pairs), the minimum short of idling the start.
  - PSUM is 8 banks; the single rotating 2-slot pool tag below
    interleaves half-B's projection/norm tiles between early gram tiles
    so half-B work overlaps the jg0 exp stream without stalling it.

Output columns of h=1 cores are rotated by 2048; the host gather undoes
it during dequantization.
"""

import numpy as np
from contextlib import ExitStack

B, S, D, HD = 4, 4096, 1024, 120
ROWS = S // 2  # rows of the score matrix each core produces
N_CORES = 8
EXP_BIAS = float(np.log(250.0) - 1.0)

_NC_CACHE = {}


def _build_nc():
    import concourse.bacc as bacc
    import concourse.tile as tile
    from concourse import mybir

    f32 = mybir.dt.float32
    fp16 = mybir.dt.float16
    u8 = mybir.dt.uint8
    AF = mybir.ActivationFunctionType
    nc = bacc.Bacc("TRN2", target_bir_lowering=False, debug=False,
                   num_devices=N_CORES)

    qt = nc.declare_dram_parameter("qt", [D, S], fp16, isOutput=False)
    mt = nc.declare_dram_parameter("mt", [D, HD], fp16, isOutput=False)
    c0 = nc.declare_dram_parameter("c0", [HD, 1], f32, isOutput=False)
    ones = nc.declare_dram_parameter("ones", [HD, HD], fp16, isOutput=False)
    eo = nc.declare_dram_parameter("eo", [ROWS, S], u8, isOutput=True)

    DC = D // 128   # 8 contraction chunks
    NCHUNK = ROWS // 128  # 16 gram row-chunks

    with ExitStack() as ctx:
        tc = ctx.enter_context(tile.TileContext(nc))
        singles = ctx.enter_context(tc.tile_pool(name="singles", bufs=1))
        qpool = ctx.enter_context(tc.tile_pool(name="qpool", bufs=1))
        work = ctx.enter_context(tc.tile_pool(name="work", bufs=1))
        epool = ctx.enter_context(tc.tile_pool(name="epool", bufs=4))
        ps = ctx.enter_context(tc.tile_pool(name="ps", bufs=2, space="PSUM"))

        # constants first in the DMA queues; mt split so it doesn't
        # serialize 0.25 MB behind one queue
        mt_sb = singles.tile([128, DC, HD], fp16, tag="mt")
        mt_r = mt.rearrange("(c p) h -> p c h", p=128)
        for cp in range(4):
            nc.sync.dma_start(out=mt_sb[:, 2 * cp:2 * cp + 2, :],
                              in_=mt_r[:, 2 * cp:2 * cp + 2, :])
        c0_sb = singles.tile([HD, 1], f32, tag="c0")
        nc.sync.dma_start(out=c0_sb[:], in_=c0[:])
        ones_sb = singles.tile([HD, HD], fp16, tag="ones")
        nc.sync.dma_start(out=ones_sb[:], in_=ones[:])

        cn = singles.tile([HD, S], fp16, tag="cn")      # both halves
        ebias = singles.tile([128, 1], f32, tag="ebias")
        nc.vector.memset(ebias[:], EXP_BIAS)

        # q tiles stay resident per half (8 x 4 KB/partition)
        q_sb = [singles.tile([128, DC, ROWS], fp16, tag=f"q{h}",
                             name=f"q{h}") for h in range(2)]

        def load_half(half):
            # [128, 512] pieces, k-major so proj k-blocks unblock in order
            for k in range(4):
                for c in range(DC):
                    nc.sync.dma_start(
                        out=q_sb[half][:, c, k * 512:(k + 1) * 512],
                        in_=qt[c * 128:(c + 1) * 128,
                               half * ROWS + k * 512:
                               half * ROWS + (k + 1) * 512])

        def proj_half(half, ct_ps):
            for k in range(4):
                for c in range(DC):
                    nc.tensor.matmul(
                        ct_ps[:, k * 512:(k + 1) * 512],
                        lhsT=mt_sb[:, c, :],
                        rhs=q_sb[half][:, c, k * 512:(k + 1) * 512],
                        start=(c == 0), stop=(c == DC - 1))

        def bias_sq(half, ct_ps):
            """DVE: ct = (CT + c0) -> fp16, sq = ct^2 (2x mode)."""
            ct_sb = work.tile([HD, ROWS], fp16, tag=f"ct{half}",
                              name=f"ct{half}")
            ctsq = work.tile([HD, ROWS], fp16, tag=f"ctsq{half}",
                             name=f"ctsq{half}")
            for s2 in range(2):
                sl = slice(s2 * 1024, (s2 + 1) * 1024)
                nc.vector.tensor_scalar_add(ct_sb[:, sl], ct_ps[:, sl],
                                            c0_sb[:])
                nc.vector.tensor_mul(ctsq[:, sl], ct_sb[:, sl], ct_sb[:, sl])
            return ct_sb, ctsq

        def ones_mm(ctsq, n_ps):
            for k in range(4):
                nc.tensor.matmul(n_ps[:, k * 512:(k + 1) * 512],
                                 lhsT=ones_sb[:],
                                 rhs=ctsq[:, k * 512:(k + 1) * 512],
                                 start=True, stop=True)

        def rsqrt_cn(half, ct_sb, n_ps):
            inv = work.tile([HD, ROWS], fp16, tag=f"inv{half}",
                            name=f"inv{half}")
            nc.scalar.activation(out=inv[:], in_=n_ps[:],
                                 func=AF.Abs_reciprocal_sqrt)
            nc.vector.tensor_mul(cn[:, half * ROWS:(half + 1) * ROWS],
                                 ct_sb[:], inv[:])

        def gram_mm(i, jg, r_ps):
            hi = cn[:, i * 128:(i + 1) * 128]
            for k in range(4):
                kk = jg * 4 + k
                nc.tensor.matmul(r_ps[:, k * 512:(k + 1) * 512],
                                 lhsT=hi, rhs=cn[:, kk * 512:(kk + 1) * 512],
                                 start=True, stop=True)

        def exp_out(i, jg, r_ps, split_dma=False):
            e_sb = epool.tile([128, 2048], u8, tag="e", name=f"e{i}_{jg}")
            nc.scalar.activation(out=e_sb[:], in_=r_ps[:], func=AF.Exp,
                                 bias=ebias[:])
            od = eo[i * 128:(i + 1) * 128, jg * 2048:(jg + 1) * 2048]
            if split_dma:  # tail latency: 2 queues instead of 1
                nc.sync.dma_start(out=od[:, :1024], in_=e_sb[:, :1024])
                nc.sync.dma_start(out=od[:, 1024:], in_=e_sb[:, 1024:])
            else:
                nc.sync.dma_start(out=od, in_=e_sb[:])

        # ---- DMA queue order: constants, half A, half B ----
        load_half(0)
        load_half(1)

        # ---- PSUM slot rotation (2 slots of [128,2048] f32 = 8 banks).
        # Tile-call order interleaves half-B tiles between early gram
        # tiles; see module docstring.
        ct_a = ps.tile([HD, ROWS], f32, tag="ps4", name="ct_a")     # T1
        n_a = ps.tile([HD, ROWS], f32, tag="ps4", name="n_a")       # T2
        r00 = ps.tile([128, 2048], f32, tag="ps4", name="r_0_0")    # T1
        r10 = ps.tile([128, 2048], f32, tag="ps4", name="r_1_0")    # T2
        ct_b = ps.tile([HD, ROWS], f32, tag="ps4", name="ct_b")     # T1
        r20 = ps.tile([128, 2048], f32, tag="ps4", name="r_2_0")    # T2
        n_b = ps.tile([HD, ROWS], f32, tag="ps4", name="n_b")       # T1
        r30 = ps.tile([128, 2048], f32, tag="ps4", name="r_3_0")    # T2

        # ---- emission: PE / DVE / ACT streams ----
        proj_half(0, ct_a)
        ct_sb_a, ctsq_a = bias_sq(0, ct_a)
        ones_mm(ctsq_a, n_a)
        rsqrt_cn(0, ct_sb_a, n_a)

        gram_mm(0, 0, r00)
        exp_out(0, 0, r00)
        gram_mm(1, 0, r10)
        exp_out(1, 0, r10)

        proj_half(1, ct_b)
        ct_sb_b, ctsq_b = bias_sq(1, ct_b)

        gram_mm(2, 0, r20)
        exp_out(2, 0, r20)

        ones_mm(ctsq_b, n_b)
        rsqrt_cn(1, ct_sb_b, n_b)

        gram_mm(3, 0, r30)
        exp_out(3, 0, r30)

        for i in range(4, NCHUNK):
            r_ps = ps.tile([128, 2048], f32, tag="ps4", name=f"r_{i}_0")
            gram_mm(i, 0, r_ps)
            exp_out(i, 0, r_ps)
        for i in range(NCHUNK):
            r_ps = ps.tile([128, 2048], f32, tag="ps4", name=f"r_{i}_1")
            gram_mm(i, 1, r_ps)
            exp_out(i, 1, r_ps, split_dma=(i >= NCHUNK - 2))

    nc.compile()
    return nc


def _get_nc():
    if "nc" not in _NC_CACHE:
        _NC_CACHE["nc"] = _build_nc()
    return _NC_CACHE["nc"]


def _make_in_maps(inputs):
    query = np.asarray(inputs["query"], dtype=np.float32)
    Wq = np.asarray(inputs["Wq"], dtype=np.float32)
    bq = np.asarray(inputs["bq"], dtype=np.float32)
    w = np.asarray(inputs["weight_tensor"], dtype=np.float32)

    w0 = w.reshape(-1)[:HD]
    mt_np = np.ascontiguousarray((w0[:, None] * Wq).T.astype(np.float16))
    c0_np = np.ascontiguousarray((w0 * bq)[:, None].astype(np.float32))
    ones_np = np.ones((HD, HD), dtype=np.float16)

    in_maps = []
    for c in range(N_CORES):
        b, h = c // 2, c % 2
        qb = query[b]
        if h:
            qb = np.concatenate([qb[ROWS:], qb[:ROWS]], axis=0)
        qt_np = np.ascontiguousarray(qb.T.astype(np.float16))
        in_maps.append({"qt": qt_np, "mt": mt_np, "c0": c0_np,
                        "ones": ones_np})
    return in_maps


def _gather(results):
    full = np.empty((B, S, S), dtype=np.float32)
    for c in range(N_CORES):
        b, h = c // 2, c % 2
        r = results[c]["eo"]
        # softmax self-normalizes from the quantized numerators
        rec = 1.0 / r.sum(axis=1, dtype=np.int64).astype(np.float32)
        deq = r.astype(np.float32) * rec[:, None]
        if h == 0:
            full[b, :ROWS] = deq
        else:
            full[b, ROWS:, ROWS:] = deq[:, :ROWS]
            full[b, ROWS:, :ROWS] = deq[:, ROWS:]
    return full


def kernel(**inputs):
    from concourse.bass_utils import run_bass_kernel_spmd

    in_maps = _make_in_maps(inputs)
    nc = _get_nc()
    res = run_bass_kernel_spmd(nc, in_maps, list(range(N_CORES))).results
    return _gather(res)


def _register_ntff_hook():
    """Register the axon NTFF profile hook that the agent image's antenv
    package lacks (see trn_boot.py) so trace=True yields exec_time_ns."""
    import sys
    import types
    try:
        import antenv.axon_hooks  # noqa: F401
        return True
    except ImportError:
        pass
    try:
        from trn_agent_boot.trn_boot import _ntff_profile_via_ctypes
        hook = _ntff_profile_via_ctypes("/opt/axon/libaxon_pjrt.so")
    except Exception:
        return False
    if hook is None:
        return False
    mod = types.ModuleType("antenv.axon_hooks")
    mod._hook = hook
    mod.get_axon_ntff_profile_hook = lambda: mod._hook
    mod.set_axon_ntff_profile_hook = lambda h: setattr(mod, "_hook", h)
    sys.modules["antenv.axon_hooks"] = mod
    import antenv
    antenv.axon_hooks = mod
    return True


def profile_once(inputs, trace_cores=None):
    """Re-run the kernel with NTFF profiling; returns max exec_time_ns."""
    import tempfile
    import concourse.bass_utils as bu

    _register_ntff_hook()
    # avoid the cloud artifact upload inside the trace path
    bu.upload_artifacts = lambda tmpdir: tmpdir

    in_maps = _make_in_maps(inputs)
    nc = _get_nc()
    tmpdir = tempfile.mkdtemp(prefix="ntff_")
    r = bu.run_bass_kernel_spmd(nc, in_maps, list(range(N_CORES)),
                                trace=True, trace_cores=trace_cores,
                                tmpdir=tmpdir)
    print(f"trace dir: {tmpdir}")
    if r.exec_time_ns is not None:
        print(f"mean exec: {r.mean_exec_time_ns} ns, "
              f"max core: {r.max_exec_time_core_id}")
    return r.exec_time_ns


# revision 15
# speedup vs baseline: 1.7093x; 1.0173x over previous
"""Cosine multi-head attention (h=1) Trainium2 kernel, v3.

Math (reference):
    context = query @ Wq.T + bq                  [B, S, HD]
    ctx     = context * weight_tensor[0]         (elementwise over HD)
    ctx_n   = ctx / max(||ctx||_2, eps)          (normalize over HD)
    scores  = ctx_n @ ctx_n.T                    [B, S, S]
    out     = softmax(scores, axis=-1)

Device strategy (8 cores, SPMD): core c handles batch b = c//2, row-half
h = c%2.  The host rotates the batch's rows so each core's own 2048 rows
come first and ships qT [D, S] in fp16; weight_tensor is folded into Wq
(M = diag(w) @ Wq, fp16) and c0 = w * bq.

Precision budget: the harness gate is rel_err < 2e-2 (abs-max / max|ref|).
A pure-fp16 pipeline (fp16 matmul inputs, fp32 PSUM accum) measures
~6e-4; quantizing the softmax numerator to uint8 adds ~2e-3.  Total
~3e-3, 7x inside the gate, while tripling matmul speed vs the fp32r /
compensated-bf16 alternatives and halving output DMA bytes.

On device:
    CT[hd, s] = sum_d M[hd, d] qT[d, s]       (PSUM [120, 2048] per half)
    ct = (CT + c0) -> fp16 (DVE), sq = ct^2 (DVE 2x)
    n  = ones[120,120] @ sq                   (PE broadcast-sum, PSUM)
    inv = n^-1/2 -> fp16   (one ACT Abs_reciprocal_sqrt per half)
    cn = ct * inv -> fp16                     (DVE 2x)
    per 128-row chunk i, col-half jg:
        R = cn[:, i-chunk].T @ cn[:, jg]      (PE, fp16, PSUM [128, 2048])
        u8 = exp(R + (ln(250) - 1))           (ACT, no accumulator)
    The exp bias maps scores [-1, 1] -> (33.6, 250.5); the uint8 cast IS
    the output write.  No row-sum is computed on device: softmax
    self-normalizes from the quantized numerators on the host,
    out = u8 / sum_j(u8) (the 250/e scale cancels; the quantization
    error this adds to the denominator is ~1e-4 relative).

Hardware lessons this version encodes (from NTFF traces):
  - one dma_start lands on ONE of the 16 DMA queues (~22.5 GB/s each),
    so every bulk transfer is chopped into [128, 512] pieces that
    round-robin across queues (4.2 MB half-q arrives in ~12 us, not 23).
  - ACT accum_out costs ~0.95 us per instruction on HW (vs 187 ns in
    the cost model) -- dropped entirely (see above).
  - Ln and Exp do NOT share an ACT table set on this toolchain: each
    switch is a 1.28 us table load.  One rsqrt per half keeps it to 4
    loads total (Sqrt/Exp pairs), the minimum short of idling the start.
  - PSUM is 8 banks; the single rotating 2-slot pool tag below
    interleaves half-B's projection/norm tiles between early gram tiles
    so half-B work overlaps the jg0 exp stream without stalling it.

Output columns of h=1 cores are rotated by 2048; the host gather undoes
it during dequantization.
"""

import numpy as np
from contextlib import ExitStack

B, S, D, HD = 4, 4096, 1024, 120
ROWS = S // 2  # rows of the score matrix each core produces
N_CORES = 8
EXP_BIAS = float(np.log(250.0) - 1.0)

_NC_CACHE = {}


def _build_nc():
    import concourse.bacc as bacc
    import concourse.tile as tile
    from concourse import mybir

    f32 = mybir.dt.float32
    fp16 = mybir.dt.float16
    u8 = mybir.dt.uint8
    AF = mybir.ActivationFunctionType
    nc = bacc.Bacc("TRN2", target_bir_lowering=False, debug=False,
                   num_devices=N_CORES)

    qt = nc.declare_dram_parameter("qt", [D, S], fp16, isOutput=False)
    mt = nc.declare_dram_parameter("mt", [D, HD], fp16, isOutput=False)
    c0 = nc.declare_dram_parameter("c0", [HD, 1], f32, isOutput=False)
    ones = nc.declare_dram_parameter("ones", [HD, HD], fp16, isOutput=False)
    eo = nc.declare_dram_parameter("eo", [ROWS, S], u8, isOutput=True)

    DC = D // 128   # 8 contraction chunks
    NCHUNK = ROWS // 128  # 16 gram row-chunks

    with ExitStack() as ctx:
        tc = ctx.enter_context(tile.TileContext(nc))
        singles = ctx.enter_context(tc.tile_pool(name="singles", bufs=1))
        qpool = ctx.enter_context(tc.tile_pool(name="qpool", bufs=1))
        work = ctx.enter_context(tc.tile_pool(name="work", bufs=1))
        epool = ctx.enter_context(tc.tile_pool(name="epool", bufs=4))
        ps = ctx.enter_context(tc.tile_pool(name="ps", bufs=2, space="PSUM"))

        cn = singles.tile([HD, S], fp16, tag="cn")      # both halves
        ebias = singles.tile([128, 1], f32, tag="ebias")
        nc.vector.memset(ebias[:], EXP_BIAS)
        junk = singles.tile([128, 1], f32, tag="junk")
        # preload the Sqrt ACT table at t=0 so rsqrtA doesn't pay the
        # 1.28 us load on the critical path (reads ebias, result unused)
        nc.scalar.activation(out=junk[:], in_=ebias[:],
                             func=AF.Abs_reciprocal_sqrt)

        # Constants go on the Pool sequencer (SWDGE): its per-DMA issue
        # cost doesn't compete with the SP/ACT issue streams below.
        c0_sb = singles.tile([HD, 1], f32, tag="c0")
        nc.sync.dma_start(out=c0_sb[:], in_=c0[:])
        mt_sb = singles.tile([128, DC, HD], fp16, tag="mt")
        mt_r = mt.rearrange("(c p) h -> p c h", p=128)
        for cp in range(4):
            nc.sync.dma_start(out=mt_sb[:, 2 * cp:2 * cp + 2, :],
                                in_=mt_r[:, 2 * cp:2 * cp + 2, :])
        ones_sb = singles.tile([HD, HD], fp16, tag="ones")
        nc.sync.dma_start(out=ones_sb[:], in_=ones[:])

        # q tiles stay resident per half (8 x 4 KB/partition)
        q_sb = [singles.tile([128, DC, ROWS], fp16, tag=f"q{h}",
                             name=f"q{h}") for h in range(2)]

        def load_half(half, issuers):
            # [128, 512] pieces, k-major so proj k-blocks unblock in
            # order.  Issue alternates across two engine sequencers: one
            # dma_start occupies its sequencer for ~0.6-1.0 us, so a
            # single sequencer serializes the whole load (the v3 trace
            # showed SP issuing back-to-back for 40+ us while every
            # engine waited on input).  Half A uses the two fast HWDGE
            # sequencers (SP + ACT); half B, which has ~15 us of slack,
            # uses SP + the slower Pool SWDGE path.
            for k in range(4):
                for c in range(DC):
                    eng = issuers[(k * DC + c) % len(issuers)]
                    eng.dma_start(
                        out=q_sb[half][:, c, k * 512:(k + 1) * 512],
                        in_=qt[c * 128:(c + 1) * 128,
                               half * ROWS + k * 512:
                               half * ROWS + (k + 1) * 512])

        def proj_half(half, ct_ps):
            for k in range(4):
                for c in range(DC):
                    nc.tensor.matmul(
                        ct_ps[:, k * 512:(k + 1) * 512],
                        lhsT=mt_sb[:, c, :],
                        rhs=q_sb[half][:, c, k * 512:(k + 1) * 512],
                        start=(c == 0), stop=(c == DC - 1))

        def bias_sq(half, ct_ps, nsub=2):
            """DVE: ct = (CT + c0) -> fp16, sq = ct^2 (2x mode)."""
            ct_sb = work.tile([HD, ROWS], fp16, tag=f"ct{half}",
                              name=f"ct{half}")
            ctsq = work.tile([HD, ROWS], fp16, tag=f"ctsq{half}",
                             name=f"ctsq{half}")
            w2 = ROWS // nsub
            for s2 in range(nsub):
                sl = slice(s2 * w2, (s2 + 1) * w2)
                nc.vector.tensor_scalar_add(ct_sb[:, sl], ct_ps[:, sl],
                                            c0_sb[:])
                nc.vector.tensor_mul(ctsq[:, sl], ct_sb[:, sl], ct_sb[:, sl])
            return ct_sb, ctsq

        def ones_mm(ctsq, n_ps):
            for k in range(4):
                nc.tensor.matmul(n_ps[:, k * 512:(k + 1) * 512],
                                 lhsT=ones_sb[:],
                                 rhs=ctsq[:, k * 512:(k + 1) * 512],
                                 start=True, stop=True)

        def rsqrt_cn(half, ct_sb, n_ps, nsub=1):
            inv = work.tile([HD, ROWS], fp16, tag=f"inv{half}",
                            name=f"inv{half}")
            w2 = ROWS // nsub
            for s2 in range(nsub):
                sl = slice(s2 * w2, (s2 + 1) * w2)
                nc.scalar.activation(out=inv[:, sl], in_=n_ps[:, sl],
                                     func=AF.Abs_reciprocal_sqrt)
                nc.vector.tensor_mul(
                    cn[:, half * ROWS + s2 * w2:half * ROWS + (s2 + 1) * w2],
                    ct_sb[:, sl], inv[:, sl])

        def gram_mm(i, jg, r_ps):
            hi = cn[:, i * 128:(i + 1) * 128]
            for k in range(4):
                kk = jg * 4 + k
                nc.tensor.matmul(r_ps[:, k * 512:(k + 1) * 512],
                                 lhsT=hi, rhs=cn[:, kk * 512:(kk + 1) * 512],
                                 start=True, stop=True)

        def exp_out(i, jg, r_ps, nout=1):
            e_sb = epool.tile([128, 2048], u8, tag="e", name=f"e{i}_{jg}")
            nc.scalar.activation(out=e_sb[:], in_=r_ps[:], func=AF.Exp,
                                 bias=ebias[:])
            od = eo[i * 128:(i + 1) * 128, jg * 2048:(jg + 1) * 2048]
            w2 = 2048 // nout  # tail latency: spread across nout queues
            for t in range(nout):
                nc.sync.dma_start(out=od[:, t * w2:(t + 1) * w2],
                                  in_=e_sb[:, t * w2:(t + 1) * w2])

        # ---- DMA issue: half A on the fast sequencers, B has slack ----
        load_half(0, [nc.sync, nc.scalar])
        load_half(1, [nc.sync, nc.scalar])

        # ---- PSUM slot rotation (2 slots of [128,2048] f32 = 8 banks).
        # Tile-call order interleaves half-B tiles between early gram
        # tiles; see module docstring.
        ct_a = ps.tile([HD, ROWS], f32, tag="ps4", name="ct_a")     # T1
        n_a = ps.tile([HD, ROWS], f32, tag="ps4", name="n_a")       # T2
        r00 = ps.tile([128, 2048], f32, tag="ps4", name="r_0_0")    # T1
        r10 = ps.tile([128, 2048], f32, tag="ps4", name="r_1_0")    # T2
        ct_b = ps.tile([HD, ROWS], f32, tag="ps4", name="ct_b")     # T1
        r20 = ps.tile([128, 2048], f32, tag="ps4", name="r_2_0")    # T2
        n_b = ps.tile([HD, ROWS], f32, tag="ps4", name="n_b")       # T1
        r30 = ps.tile([128, 2048], f32, tag="ps4", name="r_3_0")    # T2

        # ---- emission: PE / DVE / ACT streams ----
        proj_half(0, ct_a)
        ct_sb_a, ctsq_a = bias_sq(0, ct_a, nsub=4)
        ones_mm(ctsq_a, n_a)
        rsqrt_cn(0, ct_sb_a, n_a, nsub=2)

        gram_mm(0, 0, r00)
        exp_out(0, 0, r00)
        gram_mm(1, 0, r10)
        exp_out(1, 0, r10)

        proj_half(1, ct_b)
        ct_sb_b, ctsq_b = bias_sq(1, ct_b)

        gram_mm(2, 0, r20)
        exp_out(2, 0, r20)

        ones_mm(ctsq_b, n_b)
        rsqrt_cn(1, ct_sb_b, n_b)

        gram_mm(3, 0, r30)
        exp_out(3, 0, r30)

        for i in range(4, NCHUNK):
            r_ps = ps.tile([128, 2048], f32, tag="ps4", name=f"r_{i}_0")
            gram_mm(i, 0, r_ps)
            exp_out(i, 0, r_ps)
        for i in range(NCHUNK):
            r_ps = ps.tile([128, 2048], f32, tag="ps4", name=f"r_{i}_1")
            gram_mm(i, 1, r_ps)
            exp_out(i, 1, r_ps, nout=(4 if i >= NCHUNK - 2 else 1))

    nc.compile()
    return nc


def _get_nc():
    if "nc" not in _NC_CACHE:
        _NC_CACHE["nc"] = _build_nc()
    return _NC_CACHE["nc"]


def _make_in_maps(inputs):
    query = np.asarray(inputs["query"], dtype=np.float32)
    Wq = np.asarray(inputs["Wq"], dtype=np.float32)
    bq = np.asarray(inputs["bq"], dtype=np.float32)
    w = np.asarray(inputs["weight_tensor"], dtype=np.float32)

    w0 = w.reshape(-1)[:HD]
    mt_np = np.ascontiguousarray((w0[:, None] * Wq).T.astype(np.float16))
    c0_np = np.ascontiguousarray((w0 * bq)[:, None].astype(np.float32))
    ones_np = np.ones((HD, HD), dtype=np.float16)

    in_maps = []
    for c in range(N_CORES):
        b, h = c // 2, c % 2
        qb = query[b]
        if h:
            qb = np.concatenate([qb[ROWS:], qb[:ROWS]], axis=0)
        qt_np = np.ascontiguousarray(qb.T.astype(np.float16))
        in_maps.append({"qt": qt_np, "mt": mt_np, "c0": c0_np,
                        "ones": ones_np})
    return in_maps


def _gather(results):
    full = np.empty((B, S, S), dtype=np.float32)
    for c in range(N_CORES):
        b, h = c // 2, c % 2
        r = results[c]["eo"]
        # softmax self-normalizes from the quantized numerators
        rec = 1.0 / r.sum(axis=1, dtype=np.int64).astype(np.float32)
        deq = r.astype(np.float32) * rec[:, None]
        if h == 0:
            full[b, :ROWS] = deq
        else:
            full[b, ROWS:, ROWS:] = deq[:, :ROWS]
            full[b, ROWS:, :ROWS] = deq[:, ROWS:]
    return full


def kernel(**inputs):
    from concourse.bass_utils import run_bass_kernel_spmd

    in_maps = _make_in_maps(inputs)
    nc = _get_nc()
    res = run_bass_kernel_spmd(nc, in_maps, list(range(N_CORES))).results
    return _gather(res)


def _register_ntff_hook():
    """Register the axon NTFF profile hook that the agent image's antenv
    package lacks (see trn_boot.py) so trace=True yields exec_time_ns."""
    import sys
    import types
    try:
        import antenv.axon_hooks  # noqa: F401
        return True
    except ImportError:
        pass
    try:
        from trn_agent_boot.trn_boot import _ntff_profile_via_ctypes
        hook = _ntff_profile_via_ctypes("/opt/axon/libaxon_pjrt.so")
    except Exception:
        return False
    if hook is None:
        return False
    mod = types.ModuleType("antenv.axon_hooks")
    mod._hook = hook
    mod.get_axon_ntff_profile_hook = lambda: mod._hook
    mod.set_axon_ntff_profile_hook = lambda h: setattr(mod, "_hook", h)
    sys.modules["antenv.axon_hooks"] = mod
    import antenv
    antenv.axon_hooks = mod
    return True


def profile_once(inputs, trace_cores=None):
    """Re-run the kernel with NTFF profiling; returns max exec_time_ns."""
    import tempfile
    import concourse.bass_utils as bu

    _register_ntff_hook()
    # avoid the cloud artifact upload inside the trace path
    bu.upload_artifacts = lambda tmpdir: tmpdir

    in_maps = _make_in_maps(inputs)
    nc = _get_nc()
    tmpdir = tempfile.mkdtemp(prefix="ntff_")
    r = bu.run_bass_kernel_spmd(nc, in_maps, list(range(N_CORES)),
                                trace=True, trace_cores=trace_cores,
                                tmpdir=tmpdir)
    print(f"trace dir: {tmpdir}")
    if r.exec_time_ns is not None:
        print(f"mean exec: {r.mean_exec_time_ns} ns, "
              f"max core: {r.max_exec_time_core_id}")
    return r.exec_time_ns
